# revision 23
# baseline (speedup 1.0000x reference)
"""Trainium2 Bass kernel for nn_NeuroKernel_56590489092176.

Math (reference):
    P = N(N+1)/2 upper-tri pairs (x[i], x[j]), j >= i, N = 2048
    h  = sigmoid(pairs @ W1.T + b1)     # [P, 128]
    h  = relu(h @ W2.T + b2)            # [P, 32]
    v  = h @ W3.T + b3                  # [P]
    K  = zeros(N, N); K[triu] = v
    out = K.T @ K

Distribution (8 cores):
    Rows split into 32 groups of 64; strip k = groups (k, 31-k) => 17
    [64 rows x 128 cols] blocks per strip; core c owns strips 2c, 2c+1
    (34 blocks, 278,528 padded pairs per core).

    NEFF 1 (MLP), per core, in 272 subrounds of 1024 pairs:
      L1   TensorE fp32r:  pre1[128f, 1024p] = W1 @ pairs   (1 row/pair)
      sig  ScalarE:        h1 = sigmoid(pre1 + b1) -> bf16
      L2t  TensorE bf16:   z[128p, 32f] = h1_chunk^T @ (W2^T |w3|) + b2|w3|
                           (pairs on PSUM partitions => 0.25 rows/pair;
                            |w3| folded into W2/b2 columns)
      stt  VectorE:        r = max(z, 0) * sign(w3)   (folds W3 + relu)
      red  VectorE:        v[p, c] = sum_f r          (grouped reduce)
      out  v-blocks to DRAM positionally [NBLK, 128, 64] fp32.

    Host: transpose v-blocks into the 2 masked K strips (+b3), fp16.

    NEFF 2 (GEMM), per core: C_c = S0^T S0 + S1^T S1 in fp16 (1 cyc/row),
    fp16 output.  Host sums the 8 partial [2048, 2048] outputs in fp32.

Self-contained: hardcodes all shapes; only needs /opt/trn_rl_repo.
"""

import sys

if "/opt/trn_rl_repo" not in sys.path:
    sys.path.insert(0, "/opt/trn_rl_repo")

import numpy as np

import concourse.bass as bass
import concourse.bacc as bacc
import concourse.mybir as mybir
import concourse.tile as tile
from concourse.bass_utils import run_bass_kernel_spmd

N = 2048
NCORES = 8
NBLK = 34            # blocks per core (2 strips x 17)
NSUB = NBLK * 8      # 1024-pair subrounds: 272
F32 = mybir.dt.float32
F32R = mybir.dt.float32r
BF16 = mybir.dt.bfloat16
F16 = mybir.dt.float16
AF = mybir.ActivationFunctionType
ALU = mybir.AluOpType
AX = mybir.AxisListType


# ----------------------------------------------------------------- host prep

def _strips_of_core(c):
    """Two strips per core; strip = (groups (k, 31-k), block list)."""
    out = []
    for k in (2 * c, 2 * c + 1):
        blocks = [(k, tj) for tj in range(k // 2, 16)]
        blocks += [(31 - k, tj) for tj in range((31 - k) // 2, 16)]
        assert len(blocks) == 17
        out.append((k, blocks))
    return out


def _host_prep(x, W1, b1, W2, b2, W3, b3):
    """Builds the 8 per-core MLP input maps."""
    import ml_dtypes

    bf16 = ml_dtypes.bfloat16
    x = np.asarray(x, np.float32)
    w3 = np.asarray(W3, np.float32)[0]                    # [32]
    aw3 = np.abs(w3)
    common = {
        "w1h": np.ascontiguousarray(W1.T.astype(np.float32)),          # [2, 128]
        "b1h": np.ascontiguousarray(b1.astype(np.float32)[:, None]),   # [128, 1]
        "w2h": np.ascontiguousarray(
            (W2.T.astype(np.float32) * aw3[None, :]).astype(bf16)
        ),                                                              # [128, 32]
        "b2h": np.ascontiguousarray(
            np.tile(b2.astype(np.float32) * aw3, 8)[None, :].astype(bf16)
        ),                                                              # [1, 256]
        "onesh": np.ones((1, 128), bf16),
        "sgnh": np.ascontiguousarray(
            np.broadcast_to(np.tile(np.sign(w3), 16)[None, :], (128, 512))
        ).astype(np.float32),                                           # [128, 512]
    }
    in_maps = []
    for c in range(NCORES):
        strips = _strips_of_core(c)
        pt = np.empty((NBLK, 4, 2, 4, 512), np.float32)
        b = 0
        for s, (k, blocks) in enumerate(strips):
            for grp, tj in blocks:
                xj = np.tile(x[128 * tj : 128 * tj + 128], 4)           # [512]
                for r in range(4):
                    for t in range(4):
                        i0 = 64 * grp + 16 * t + 4 * r
                        pt[b, r, 0, t] = np.repeat(x[i0 : i0 + 4], 128)
                        pt[b, r, 1, t] = xj
                b += 1
        assert b == NBLK
        m = dict(common)
        m["pt"] = pt
        in_maps.append(m)
    return in_maps


# ------------------------------------------------------- NEFF 1: the MLP

def build_nc():
    nc = bacc.Bacc("TRN2", target_bir_lowering=False, debug=False)

    ptd = nc.dram_tensor("pt", [NBLK, 4, 2, 4, 512], F32R, kind="ExternalInput")
    w1d = nc.dram_tensor("w1h", [2, 128], F32R, kind="ExternalInput")
    b1d = nc.dram_tensor("b1h", [128, 1], F32, kind="ExternalInput")
    w2d = nc.dram_tensor("w2h", [128, 32], BF16, kind="ExternalInput")
    b2d = nc.dram_tensor("b2h", [1, 256], BF16, kind="ExternalInput")
    onesd = nc.dram_tensor("onesh", [1, 128], BF16, kind="ExternalInput")
    sgnd = nc.dram_tensor("sgnh", [128, 512], F32, kind="ExternalInput")
    kbd = nc.dram_tensor("kblk", [NBLK, 128, 64], F32, kind="ExternalOutput")

    with tile.TileContext(nc) as tc:
        with (
            tc.tile_pool(name="consts", bufs=1) as consts,
            tc.tile_pool(name="ptp", bufs=3) as ptp,
            tc.tile_pool(name="h1p", bufs=3) as h1p,
            tc.tile_pool(name="rp", bufs=3) as rp,
            tc.tile_pool(name="vp", bufs=3) as vp,
            tc.tile_pool(name="pre1p", bufs=3, space="PSUM") as pre1p,
            tc.tile_pool(name="zpp", bufs=2, space="PSUM") as zpp,
        ):
            w1sb = consts.tile([128, 128], F32R)
            for r in range(4):
                (nc.sync if r < 2 else nc.gpsimd).dma_start(
                    w1sb[32 * r : 32 * r + 2, 0:128], w1d.ap()
                )
            b1sb = consts.tile([128, 1], F32)
            nc.sync.dma_start(b1sb[:], b1d.ap())
            w2sb = consts.tile([128, 32], BF16)
            nc.gpsimd.dma_start(w2sb[:], w2d.ap())
            b2sb = consts.tile([1, 256], BF16)
            nc.gpsimd.dma_start(b2sb[:], b2d.ap())
            onesb = consts.tile([1, 128], BF16)
            nc.gpsimd.dma_start(onesb[:], onesd.ap())
            sgnsb = consts.tile([128, 512], F32)
            nc.gpsimd.dma_start(sgnsb[:], sgnd.ap())

            st = {}

            def load_pt_part(blk, r):
                # one [2, 2048] r-group per call; split across SP/Pool queues
                # so no queue backs up at block boundaries
                if r == 0:
                    st[("pt", blk)] = ptp.tile([128, 2048], F32R, name="ptsb")
                ptsb = st[("pt", blk)]
                (nc.sync if r < 2 else nc.gpsimd).dma_start(
                    ptsb[32 * r : 32 * r + 2, 0:2048],
                    ptd.ap()[blk : blk + 1, r : r + 1].rearrange(
                        "a b d t e -> (a b) d (t e)"
                    ).squeeze(0),
                )

            def stage_l1(i):
                blk, sub = divmod(i, 8)
                t, h = divmod(sub, 2)
                ptsb = st[("pt", blk)]
                pre1 = pre1p.tile([128, 1024], F32, name="pre1")
                for rho in range(2):
                    r = 2 * h + rho
                    nc.tensor.matmul(
                        pre1[:, 512 * rho : 512 * (rho + 1)],
                        lhsT=w1sb[32 * r : 32 * r + 2, 0:128],
                        rhs=ptsb[32 * r : 32 * r + 2, 512 * t : 512 * (t + 1)],
                        start=True,
                        stop=True,
                        tile_position=(32 * r, 0),
                    )
                st[("pre1", i)] = pre1
                if 2 <= sub < 6 and blk + 1 < NBLK:
                    load_pt_part(blk + 1, sub - 2)  # prefetch next block
                if sub == 7:
                    st.pop(("pt", blk))

            def stage_sig(i):
                pre1 = st.pop(("pre1", i))
                h1 = h1p.tile([128, 1024], BF16)
                nc.scalar.activation(
                    h1[:], pre1[:, 0:1024], AF.Sigmoid, bias=b1sb[:, 0:1], scale=1.0
                )
                st[("h1", i)] = h1

            def stage_l2(i):
                h1 = st.pop(("h1", i))
                p, odd = divmod(i, 2)
                if odd == 0:
                    st[("z", p)] = zpp.tile([128, 512], F32, name="zps")
                zps = st[("z", p)]
                base = 256 * odd
                nc.tensor.matmul(
                    zps[:, base : base + 256],
                    lhsT=onesb[0:1, 0:128],
                    rhs=b2sb[0:1, 0:256],
                    start=True,
                    stop=False,
                    skip_group_check=True,
                )
                for cc in range(8):
                    nc.tensor.matmul(
                        zps[:, base + 32 * cc : base + 32 * cc + 32],
                        lhsT=h1[:, 128 * cc : 128 * (cc + 1)],
                        rhs=w2sb[:, 0:32],
                        start=False,
                        stop=True,
                        skip_group_check=True,
                    )

            def stage_red(p):
                # one stt+reduce per PAIR of subrounds (i = 2p, 2p+1)
                blk, pr = divmod(p, 4)
                zps = st.pop(("z", p))
                if pr == 0:
                    st[("v", blk)] = vp.tile([128, 64], F32, name="vblk")
                v = st[("v", blk)]
                rsb = rp.tile([128, 512], F32)
                nc.vector.scalar_tensor_tensor(
                    rsb[:],
                    zps[:, 0:512],
                    0.0,
                    sgnsb[:, 0:512],
                    op0=ALU.max,
                    op1=ALU.mult,
                )
                nc.vector.tensor_reduce(
                    v[:, 16 * pr : 16 * pr + 16],
                    rsb[:].rearrange("p (c f) -> p c f", f=32),
                    axis=AX.X,
                    op=ALU.add,
                )
                if pr == 3:
                    v = st.pop(("v", blk))
                    nc.sync.dma_start(kbd.ap()[blk : blk + 1].squeeze(0), v[:])

            for r in range(4):
                load_pt_part(0, r)
            for i in range(NSUB + 3):
                if i < NSUB:
                    stage_l1(i)
                if 1 <= i < NSUB + 1:
                    stage_sig(i - 1)
                if 2 <= i < NSUB + 2:
                    stage_l2(i - 2)
                if i >= 3 and (i - 2) % 2 == 1:
                    stage_red((i - 3) // 2)

    nc.compile()
    return nc


# ------------------------------------------------------- NEFF 2: the GEMM

def build_nc_gemm():
    """C = S0^T S0 + S1^T S1, upper 512-block-triangle only (C symmetric;
    host mirrors).  Row-tile a covers cols [512*(a//4), 2048)."""
    nc = bacc.Bacc("TRN2", target_bir_lowering=False, debug=False)
    ksd = nc.dram_tensor("kst", [2, 128, N], F16, kind="ExternalInput")
    cpd = nc.dram_tensor("cpart", [N, N], F16, kind="ExternalOutput")

    with tile.TileContext(nc) as tc:
        with (
            tc.tile_pool(name="gemm", bufs=1) as gemm,
            tc.tile_pool(name="psp", bufs=2, space="PSUM") as psp,
            tc.tile_pool(name="csbp", bufs=3) as csbp,
        ):
            warm = gemm.tile([128, 512], F16, tag="warm")
            nc.vector.memset(warm[:], 0.0)
            strips = []
            for s in range(2):
                stile = gemm.tile([128, 2048], F16, tag=f"strip{s}")
                (nc.sync if s == 0 else nc.scalar).dma_start(
                    stile[:], ksd.ap()[s : s + 1].squeeze(0)
                )
                strips.append(stile)
            # ramp the PE p-state while the strip DMAs are in flight
            # (warmup matmuls write into the first C psum tile, overwritten
            # by the real accumulation below)
            cps0 = psp.tile([128, 2048], F32, name="cps")
            for _ in range(8):
                nc.tensor.matmul(
                    cps0[:, 0:512], lhsT=warm[:, 0:128], rhs=warm[:, 0:512],
                    start=True, stop=True, skip_group_check=True,
                )

            for a in range(16):
                j0 = a // 4
                cps = cps0 if a == 0 else psp.tile([128, 2048], F32, name="cps")
                for j in range(j0, 4):
                    nc.tensor.matmul(
                        cps[:, 512 * j : 512 * (j + 1)],
                        lhsT=strips[0][:, 128 * a : 128 * a + 128],
                        rhs=strips[0][:, 512 * j : 512 * (j + 1)],
                        start=True,
                        stop=False,
                        skip_group_check=True,
                    )
                    nc.tensor.matmul(
                        cps[:, 512 * j : 512 * (j + 1)],
                        lhsT=strips[1][:, 128 * a : 128 * a + 128],
                        rhs=strips[1][:, 512 * j : 512 * (j + 1)],
                        start=False,
                        stop=True,
                        skip_group_check=True,
                    )
                w = 2048 - 512 * j0
                csb = csbp.tile([128, 2048], F16)
                if a % 2 == 0:
                    nc.vector.tensor_copy(csb[:, 0:w], cps[:, 512 * j0 : 2048])
                else:
                    nc.scalar.copy(csb[:, 0:w], cps[:, 512 * j0 : 2048])
                nc.sync.dma_start(
                    cpd.ap()[128 * a : 128 * a + 128, 512 * j0 : 2048],
                    csb[:, 0:w],
                )

    nc.compile()
    return nc


_NC_MLP = None
_NC_GEMM = None


def _get_nc():
    global _NC_MLP
    if _NC_MLP is None:
        _NC_MLP = build_nc()
    return _NC_MLP


def _get_nc_gemm():
    global _NC_GEMM
    if _NC_GEMM is None:
        _NC_GEMM = build_nc_gemm()
    return _NC_GEMM


def _assemble_strips(c, kblk, b3):
    """Host: v-blocks [NBLK, 128, 64] -> 2 masked fp16 K strips (+b3).

    v[p, col] of block b holds pair (i = 64*grp + col, j = 128*tj + p);
    the strip row for i is 64*half + col.
    """
    kst = np.zeros((2, 128, N), np.float32)
    b = 0
    for s, (k, blocks) in enumerate(_strips_of_core(c)):
        for grp, tj in blocks:
            half = 0 if grp == k else 1
            kst[s, 64 * half : 64 * half + 64, 128 * tj : 128 * tj + 128] = kblk[b].T
            b += 1
    kst += b3
    for s, k in enumerate((2 * c, 2 * c + 1)):
        rows = np.concatenate(
            [64 * k + np.arange(64), 64 * (31 - k) + np.arange(64)]
        )
        kst[s] *= np.arange(N)[None, :] >= rows[:, None]
    return kst.astype(np.float16)


def kernel(x, W1, b1, W2, b2, W3, b3):
    in_maps = _host_prep(
        np.asarray(x), np.asarray(W1), np.asarray(b1), np.asarray(W2),
        np.asarray(b2), np.asarray(W3), np.asarray(b3),
    )
    res_a = run_bass_kernel_spmd(_get_nc(), in_maps, core_ids=list(range(NCORES)))
    b3f = float(np.asarray(b3, np.float32)[0])
    gemm_maps = [
        {"kst": _assemble_strips(c, res_a.results[c]["kblk"], b3f)}
        for c in range(NCORES)
    ]
    res_b = run_bass_kernel_spmd(
        _get_nc_gemm(), gemm_maps, core_ids=list(range(NCORES))
    )
    out = np.zeros((N, N), np.float32)
    for c in range(NCORES):
        out += res_b.results[c]["cpart"].astype(np.float32)
    # only the upper 512-block-triangle was computed; zero the rest,
    # mirror, and halve the double-counted diagonal 512-blocks
    for bi in range(4):
        out[512 * bi : 512 * (bi + 1), : 512 * bi] = 0.0
    out = out + out.T
    for bi in range(4):
        sl = slice(512 * bi, 512 * (bi + 1))
        out[sl, sl] *= 0.5
    return out


# revision 24
# speedup vs baseline: 1.0541x; 1.0541x over previous
"""Trainium2 Bass kernel for nn_NeuroKernel_56590489092176.

Math (reference):
    P = N(N+1)/2 upper-tri pairs (x[i], x[j]), j >= i, N = 2048
    h  = sigmoid(pairs @ W1.T + b1)     # [P, 128]
    h  = relu(h @ W2.T + b2)            # [P, 32]
    v  = h @ W3.T + b3                  # [P]
    K  = zeros(N, N); K[triu] = v
    out = K.T @ K

Distribution (8 cores):
    Rows split into 32 groups of 64; strip k = groups (k, 31-k) => 17
    [64 rows x 128 cols] blocks per strip; core c owns strips 2c, 2c+1
    (34 blocks, 278,528 padded pairs per core).

    NEFF 1 (MLP), per core, in 272 subrounds of 1024 pairs:
      L1   TensorE fp32r:  pre1[128f, 1024p] = W1 @ pairs   (1 row/pair)
      sig  ScalarE:        h1 = sigmoid(pre1 + b1) -> bf16
      L2t  TensorE bf16:   z[128p, 32f] = h1_chunk^T @ (W2^T |w3|) + b2|w3|
                           (pairs on PSUM partitions => 0.25 rows/pair;
                            |w3| folded into W2/b2 columns)
      stt  VectorE:        r = max(z, 0) * sign(w3)   (folds W3 + relu)
      red  VectorE:        v[p, c] = sum_f r          (grouped reduce)
      out  v-blocks to DRAM positionally [NBLK, 128, 64] fp32.

    Host: transpose v-blocks into the 2 masked K strips (+b3), fp16.

    NEFF 2 (GEMM), per core: C_c = S0^T S0 + S1^T S1 in fp16 (1 cyc/row),
    fp16 output.  Host sums the 8 partial [2048, 2048] outputs in fp32.

Self-contained: hardcodes all shapes; only needs /opt/trn_rl_repo.
"""

import sys

if "/opt/trn_rl_repo" not in sys.path:
    sys.path.insert(0, "/opt/trn_rl_repo")

import numpy as np

import concourse.bass as bass
import concourse.bacc as bacc
import concourse.mybir as mybir
import concourse.tile as tile
from concourse.bass_utils import run_bass_kernel_spmd

N = 2048
NCORES = 8
NBLK = 34            # blocks per core (2 strips x 17)
NSUB = NBLK * 8      # 1024-pair subrounds: 272
F32 = mybir.dt.float32
F32R = mybir.dt.float32r
BF16 = mybir.dt.bfloat16
F16 = mybir.dt.float16
AF = mybir.ActivationFunctionType
ALU = mybir.AluOpType
AX = mybir.AxisListType


# ----------------------------------------------------------------- host prep

def _strips_of_core(c):
    """Two strips per core; strip = (groups (k, 31-k), block list)."""
    out = []
    for k in (2 * c, 2 * c + 1):
        blocks = [(k, tj) for tj in range(k // 2, 16)]
        blocks += [(31 - k, tj) for tj in range((31 - k) // 2, 16)]
        assert len(blocks) == 17
        out.append((k, blocks))
    return out


def _host_prep(x, W1, b1, W2, b2, W3, b3):
    """Builds the 8 per-core MLP input maps."""
    import ml_dtypes

    bf16 = ml_dtypes.bfloat16
    x = np.asarray(x, np.float32)
    w3 = np.asarray(W3, np.float32)[0]                    # [32]
    aw3 = np.abs(w3)
    common = {
        "w1h": np.ascontiguousarray(W1.T.astype(np.float32)),          # [2, 128]
        "b1h": np.ascontiguousarray(b1.astype(np.float32)[:, None]),   # [128, 1]
        "w2h": np.ascontiguousarray(
            (W2.T.astype(np.float32) * aw3[None, :]).astype(bf16)
        ),                                                              # [128, 32]
        "b2h": np.ascontiguousarray(
            np.tile(b2.astype(np.float32) * aw3, 12)[None, :].astype(bf16)
        ),                                                              # [1, 384]
        "onesh": np.ones((1, 128), bf16),
        "sgnh": np.ascontiguousarray(
            np.broadcast_to(np.tile(np.sign(w3), 16)[None, :], (128, 512))
        ).astype(np.float32),                                           # [128, 512]
    }
    in_maps = []
    for c in range(NCORES):
        strips = _strips_of_core(c)
        pt = np.empty((NBLK, 4, 2, 4, 512), np.float32)
        b = 0
        for s, (k, blocks) in enumerate(strips):
            for grp, tj in blocks:
                xj = np.tile(x[128 * tj : 128 * tj + 128], 4)           # [512]
                for r in range(4):
                    for t in range(4):
                        i0 = 64 * grp + 16 * t + 4 * r
                        pt[b, r, 0, t] = np.repeat(x[i0 : i0 + 4], 128)
                        pt[b, r, 1, t] = xj
                b += 1
        assert b == NBLK
        m = dict(common)
        m["pt"] = pt
        in_maps.append(m)
    return in_maps


# ------------------------------------------------------- NEFF 1: the MLP

def build_nc():
    nc = bacc.Bacc("TRN2", target_bir_lowering=False, debug=False)

    ptd = nc.dram_tensor("pt", [NBLK, 4, 2, 4, 512], F32R, kind="ExternalInput")
    w1d = nc.dram_tensor("w1h", [2, 128], F32R, kind="ExternalInput")
    b1d = nc.dram_tensor("b1h", [128, 1], F32, kind="ExternalInput")
    w2d = nc.dram_tensor("w2h", [128, 32], BF16, kind="ExternalInput")
    b2d = nc.dram_tensor("b2h", [1, 384], BF16, kind="ExternalInput")
    onesd = nc.dram_tensor("onesh", [1, 128], BF16, kind="ExternalInput")
    sgnd = nc.dram_tensor("sgnh", [128, 512], F32, kind="ExternalInput")
    kbd = nc.dram_tensor("kblk", [NBLK, 128, 64], F32, kind="ExternalOutput")

    # flat 512-pair units: u -> (blk = u//16, r = (u%16)//4, t = u%4)
    # subrounds of 3 units (last one ragged): one [128, 1536] sigmoid each
    NU = NBLK * 16                       # 544
    subs = [list(range(s, min(s + 3, NU))) for s in range(0, NU, 3)]
    NS = len(subs)                       # 182

    def u_rt(u):
        return (u % 16) // 4, u % 4

    with tile.TileContext(nc) as tc:
        with (
            tc.tile_pool(name="consts", bufs=1) as consts,
            tc.tile_pool(name="ptp", bufs=3) as ptp,
            tc.tile_pool(name="h1p", bufs=3) as h1p,
            tc.tile_pool(name="rp", bufs=3) as rp,
            tc.tile_pool(name="vp", bufs=3) as vp,
            tc.tile_pool(name="pre1p", bufs=2, space="PSUM") as pre1p,
            tc.tile_pool(name="zpp", bufs=2, space="PSUM") as zpp,
        ):
            w1sb = consts.tile([128, 128], F32R)
            for r in range(4):
                (nc.sync if r < 2 else nc.gpsimd).dma_start(
                    w1sb[32 * r : 32 * r + 2, 0:128], w1d.ap()
                )
            b1sb = consts.tile([128, 1], F32)
            nc.sync.dma_start(b1sb[:], b1d.ap())
            w2sb = consts.tile([128, 32], BF16)
            nc.gpsimd.dma_start(w2sb[:], w2d.ap())
            b2sb = consts.tile([1, 384], BF16)
            nc.gpsimd.dma_start(b2sb[:], b2d.ap())
            onesb = consts.tile([1, 128], BF16)
            nc.gpsimd.dma_start(onesb[:], onesd.ap())
            sgnsb = consts.tile([128, 512], F32)
            nc.gpsimd.dma_start(sgnsb[:], sgnd.ap())

            st = {}

            def load_pt_part(blk, r):
                if r == 0:
                    st[("pt", blk)] = ptp.tile([128, 2048], F32R, name="ptsb")
                ptsb = st[("pt", blk)]
                (nc.sync if r < 2 else nc.gpsimd).dma_start(
                    ptsb[32 * r : 32 * r + 2, 0:2048],
                    ptd.ap()[blk : blk + 1, r : r + 1].rearrange(
                        "a b d t e -> (a b) d (t e)"
                    ).squeeze(0),
                )

            def stage_l1(s):
                us = subs[s]
                pre1 = pre1p.tile([128, 1536], F32, name="pre1")
                for q, u in enumerate(us):
                    blk, (r, t) = u // 16, u_rt(u)
                    ptsb = st[("pt", blk)]
                    nc.tensor.matmul(
                        pre1[:, 512 * q : 512 * (q + 1)],
                        lhsT=w1sb[32 * r : 32 * r + 2, 0:128],
                        rhs=ptsb[32 * r : 32 * r + 2, 512 * t : 512 * (t + 1)],
                        start=True,
                        stop=True,
                        tile_position=(32 * r, 0),
                    )
                    if u % 16 in (4, 5, 6, 7) and blk + 1 < NBLK:
                        load_pt_part(blk + 1, u % 16 - 4)
                st[("pre1", s)] = pre1

            def stage_sig(s):
                w = 512 * len(subs[s])
                pre1 = st.pop(("pre1", s))
                h1 = h1p.tile([128, 1536], BF16, name="h1")
                nc.scalar.activation(
                    h1[:, 0:w], pre1[:, 0:w], AF.Sigmoid, bias=b1sb[:, 0:1],
                    scale=1.0,
                )
                st[("h1", s)] = h1

            def stage_l2(s):
                us = subs[s]
                w = 512 * len(us)
                zw = 128 * len(us)
                h1 = st.pop(("h1", s))
                zps = zpp.tile([128, 384], F32, name="zps")
                nc.tensor.matmul(
                    zps[:, 0:zw],
                    lhsT=onesb[0:1, 0:128],
                    rhs=b2sb[0:1, 0:zw],
                    start=True,
                    stop=False,
                    skip_group_check=True,
                )
                for cc in range(w // 128):
                    nc.tensor.matmul(
                        zps[:, 32 * cc : 32 * cc + 32],
                        lhsT=h1[:, 128 * cc : 128 * (cc + 1)],
                        rhs=w2sb[:, 0:32],
                        start=False,
                        stop=True,
                        skip_group_check=True,
                    )
                st[("z", s)] = zps

            def stage_red(s):
                us = subs[s]
                zw = 128 * len(us)
                zps = st.pop(("z", s))
                rsb = rp.tile([128, 384], F32, name="rsb")
                nc.vector.scalar_tensor_tensor(
                    rsb[:, 0:zw],
                    zps[:, 0:zw],
                    0.0,
                    sgnsb[:, 0:zw],
                    op0=ALU.max,
                    op1=ALU.mult,
                )
                for q, u in enumerate(us):
                    blk, (r, t) = u // 16, u_rt(u)
                    if u % 16 == 0:
                        st[("v", blk)] = vp.tile([128, 64], F32, name="vblk")
                    v = st[("v", blk)]
                    nc.vector.tensor_reduce(
                        v[:, 16 * t + 4 * r : 16 * t + 4 * r + 4],
                        rsb[:, 128 * q : 128 * (q + 1)].rearrange(
                            "p (c f) -> p c f", f=32
                        ),
                        axis=AX.X,
                        op=ALU.add,
                    )
                    if u % 16 == 15:
                        v = st.pop(("v", blk))
                        nc.sync.dma_start(
                            kbd.ap()[blk : blk + 1].squeeze(0), v[:]
                        )

            for r in range(4):
                load_pt_part(0, r)
            for i in range(NS + 3):
                if i < NS:
                    stage_l1(i)
                if 1 <= i < NS + 1:
                    stage_sig(i - 1)
                if 2 <= i < NS + 2:
                    stage_l2(i - 2)
                if 3 <= i:
                    stage_red(i - 3)

    nc.compile()
    return nc


# ------------------------------------------------------- NEFF 2: the GEMM

def build_nc_gemm():
    """C = S0^T S0 + S1^T S1, upper 512-block-triangle only (C symmetric;
    host mirrors).  Row-tile a covers cols [512*(a//4), 2048)."""
    nc = bacc.Bacc("TRN2", target_bir_lowering=False, debug=False)
    ksd = nc.dram_tensor("kst", [2, 128, N], F16, kind="ExternalInput")
    cpd = nc.dram_tensor("cpart", [N, N], F16, kind="ExternalOutput")

    with tile.TileContext(nc) as tc:
        with (
            tc.tile_pool(name="gemm", bufs=1) as gemm,
            tc.tile_pool(name="psp", bufs=2, space="PSUM") as psp,
            tc.tile_pool(name="csbp", bufs=3) as csbp,
        ):
            warm = gemm.tile([128, 512], F16, tag="warm")
            nc.vector.memset(warm[:], 0.0)
            strips = []
            for s in range(2):
                stile = gemm.tile([128, 2048], F16, tag=f"strip{s}")
                (nc.sync if s == 0 else nc.scalar).dma_start(
                    stile[:], ksd.ap()[s : s + 1].squeeze(0)
                )
                strips.append(stile)
            # ramp the PE p-state while the strip DMAs are in flight
            # (warmup matmuls write into the first C psum tile, overwritten
            # by the real accumulation below)
            cps0 = psp.tile([128, 2048], F32, name="cps")
            for _ in range(8):
                nc.tensor.matmul(
                    cps0[:, 0:512], lhsT=warm[:, 0:128], rhs=warm[:, 0:512],
                    start=True, stop=True, skip_group_check=True,
                )

            for a in range(16):
                j0 = a // 4
                cps = cps0 if a == 0 else psp.tile([128, 2048], F32, name="cps")
                for j in range(j0, 4):
                    nc.tensor.matmul(
                        cps[:, 512 * j : 512 * (j + 1)],
                        lhsT=strips[0][:, 128 * a : 128 * a + 128],
                        rhs=strips[0][:, 512 * j : 512 * (j + 1)],
                        start=True,
                        stop=False,
                        skip_group_check=True,
                    )
                    nc.tensor.matmul(
                        cps[:, 512 * j : 512 * (j + 1)],
                        lhsT=strips[1][:, 128 * a : 128 * a + 128],
                        rhs=strips[1][:, 512 * j : 512 * (j + 1)],
                        start=False,
                        stop=True,
                        skip_group_check=True,
                    )
                w = 2048 - 512 * j0
                csb = csbp.tile([128, 2048], F16)
                if a % 2 == 0:
                    nc.vector.tensor_copy(csb[:, 0:w], cps[:, 512 * j0 : 2048])
                else:
                    nc.scalar.copy(csb[:, 0:w], cps[:, 512 * j0 : 2048])
                nc.sync.dma_start(
                    cpd.ap()[128 * a : 128 * a + 128, 512 * j0 : 2048],
                    csb[:, 0:w],
                )

    nc.compile()
    return nc


_NC_MLP = None
_NC_GEMM = None


def _get_nc():
    global _NC_MLP
    if _NC_MLP is None:
        _NC_MLP = build_nc()
    return _NC_MLP


def _get_nc_gemm():
    global _NC_GEMM
    if _NC_GEMM is None:
        _NC_GEMM = build_nc_gemm()
    return _NC_GEMM


def _assemble_strips(c, kblk, b3):
    """Host: v-blocks [NBLK, 128, 64] -> 2 masked fp16 K strips (+b3).

    v[p, col] of block b holds pair (i = 64*grp + col, j = 128*tj + p);
    the strip row for i is 64*half + col.
    """
    kst = np.zeros((2, 128, N), np.float32)
    b = 0
    for s, (k, blocks) in enumerate(_strips_of_core(c)):
        for grp, tj in blocks:
            half = 0 if grp == k else 1
            kst[s, 64 * half : 64 * half + 64, 128 * tj : 128 * tj + 128] = kblk[b].T
            b += 1
    kst += b3
    for s, k in enumerate((2 * c, 2 * c + 1)):
        rows = np.concatenate(
            [64 * k + np.arange(64), 64 * (31 - k) + np.arange(64)]
        )
        kst[s] *= np.arange(N)[None, :] >= rows[:, None]
    return kst.astype(np.float16)


def kernel(x, W1, b1, W2, b2, W3, b3):
    in_maps = _host_prep(
        np.asarray(x), np.asarray(W1), np.asarray(b1), np.asarray(W2),
        np.asarray(b2), np.asarray(W3), np.asarray(b3),
    )
    res_a = run_bass_kernel_spmd(_get_nc(), in_maps, core_ids=list(range(NCORES)))
    b3f = float(np.asarray(b3, np.float32)[0])
    gemm_maps = [
        {"kst": _assemble_strips(c, res_a.results[c]["kblk"], b3f)}
        for c in range(NCORES)
    ]
    res_b = run_bass_kernel_spmd(
        _get_nc_gemm(), gemm_maps, core_ids=list(range(NCORES))
    )
    out = np.zeros((N, N), np.float32)
    for c in range(NCORES):
        out += res_b.results[c]["cpart"].astype(np.float32)
    # only the upper 512-block-triangle was computed; zero the rest,
    # mirror, and halve the double-counted diagonal 512-blocks
    for bi in range(4):
        out[512 * bi : 512 * (bi + 1), : 512 * bi] = 0.0
    out = out + out.T
    for bi in range(4):
        sl = slice(512 * bi, 512 * (bi + 1))
        out[sl, sl] *= 0.5
    return out
